# revision 9
# baseline (speedup 1.0000x reference)
"""Distributed GQA attention kernel for 8 Trainium2 NeuronCores.

Contract: kernel(**inputs) takes the FULL unsharded inputs of the reference
nn.Module (x, Wq, bq, Wk, bk, Wv, bv, Wo, bo) and returns the FULL
[B, T, E] float32 output.

Sharding: tensor-parallel over kv heads. Core c owns kv head c and q heads
4c..4c+3. Per iteration it makes ONE pass over x: each 512-token chunk gets
the k|v projection (stacked in one PSUM chain), RoPE, an xbar DMA-transpose
of v to token-major, and the q projection + RoPE. Attention for a (batch,
512-query) unit is emitted interleaved with later projection chunks so the
Activation engine (exp is the per-batch long pole) starts early and the PE
never starves. Scores run as concurrent 64x128 row-tiles (the two heads of
a GQA pair occupy PE row groups 0/64 via tile_position auto-derivation).
PV uses the transposed orientation - stationary = probabilities [128k,128q],
moving = v++ones [128k,65] - so the array is fully utilized and the softmax
denominator lands as column 64 of the accumulator; the per-query reciprocal
is then a natural per-partition scalar multiply. y is flipped back to
head-major with xbar DMA-transposes (no PSUM/PE cost), AllToAll'd in two
half-size collectives per batch, and o_proj runs with the full Wo on this
core's disjoint 512-token slice, interleaved with the tail of batch-1
attention to cover the collective windows. Weights/tables load once outside
the repeat loop. The host only slices/concatenates.
"""

from contextlib import ExitStack

import concourse.bass as bass
import concourse.mybir as mybir
import concourse.tile as tile
from concourse import bacc

F32 = mybir.dt.float32
BF16 = mybir.dt.bfloat16
AF = mybir.ActivationFunctionType
ALU = mybir.AluOpType

N_CORES = 8


def build(B=2, T=2048, E=2048, D=64, HQ_PER_CORE=4, repeat=1,
          no_collective=False):
    BT = B * T
    DQ = HQ_PER_CORE * D          # 256
    TS = T // N_CORES             # per-core token slice per batch
    KC = E // 128                 # contraction chunks
    NB = BT // 512                # projection chunks
    QC = T // 512                 # q chunks per batch

    nc = bacc.Bacc("TRN2", target_bir_lowering=False, debug=False,
                   num_devices=N_CORES)

    xT = nc.dram_tensor("xT", [E, BT], BF16, kind="ExternalInput").ap()
    wqT = nc.dram_tensor("wqT", [E, DQ], BF16, kind="ExternalInput").ap()
    wkT = nc.dram_tensor("wkT", [E, D], BF16, kind="ExternalInput").ap()
    wvT = nc.dram_tensor("wvT", [E, D], BF16, kind="ExternalInput").ap()
    bq = nc.dram_tensor("bq", [DQ, 1], F32, kind="ExternalInput").ap()
    bkv = nc.dram_tensor("bkv", [128, 1], F32, kind="ExternalInput").ap()
    woT = nc.dram_tensor("woT", [E, E], BF16, kind="ExternalInput").ap()
    bo = nc.dram_tensor("bo", [1, E], BF16, kind="ExternalInput").ap()
    cos_d = nc.dram_tensor("cosf", [128, BT], BF16, kind="ExternalInput").ap()
    sin_d = nc.dram_tensor("sinm", [128, BT], BF16, kind="ExternalInput").ap()
    mask_d = nc.dram_tensor("mask", [128, 128], BF16, kind="ExternalInput").ap()
    out = nc.dram_tensor("out", [B * TS, E], BF16, kind="ExternalOutput").ap()

    v = dict(no_collective=no_collective, B=B, T=T, E=E, D=D, HQ=HQ_PER_CORE,
             BT=BT, DQ=DQ, TS=TS, KC=KC, NB=NB, QC=QC,
             xT=xT, wqT=wqT, wkT=wkT, wvT=wvT, bq=bq, bkv=bkv,
             woT=woT, bo=bo, cos_d=cos_d, sin_d=sin_d, mask_d=mask_d,
             out=out)

    with tile.TileContext(nc) as tc:
        with ExitStack() as ctx:
            # ---- persistent SBUF: weights/tables loaded once ----
            pers = ctx.enter_context(tc.tile_pool(name="pers", bufs=1))
            wq_sb = pers.tile([128, KC, DQ], BF16, tag="wq", name="wq_sb")
            wkv_sb = pers.tile([128, KC, 2 * D], BF16, tag="wkv", name="wkv_sb")
            bq_sb = pers.tile([128, DQ // 128], F32, tag="bq", name="bq_sb")
            bkv_sb = pers.tile([128, 1], F32, tag="bkv", name="bkv_sb")
            wo_sb = pers.tile([128, KC, E], BF16, tag="wo", name="wo_sb")
            bo_sb = pers.tile([1, E], BF16, tag="bo", name="bo_sb")
            bo_bc = pers.tile([128, E], BF16, tag="bo_bc", name="bo_bc")
            mask_sb = pers.tile([128, 128], BF16, tag="mask", name="mask_sb")
            cos_sb = pers.tile([128, BT], BF16, tag="cos", name="cos_sb")
            sin_sb = pers.tile([128, BT], BF16, tag="sin", name="sin_sb")

            qT_sb = pers.tile([128, 2, BT], BF16, tag="qT", name="qT_sb")
            kT_sb = pers.tile([128, BT], BF16, tag="kT", name="kT_sb")
            vaug_sb = pers.tile([128, BT // 128, D + 1], BF16, tag="vaug",
                                name="vaug_sb")

            nc.sync.dma_start(wkv_sb[:, :, 0:D],
                              wkT.rearrange("(kc p) m -> p kc m", p=128))
            nc.sync.dma_start(wkv_sb[:, :, D:2 * D],
                              wvT.rearrange("(kc p) m -> p kc m", p=128))
            nc.sync.dma_start(bkv_sb[:], bkv[:])
            nc.sync.dma_start(wq_sb[:], wqT.rearrange("(kc p) m -> p kc m", p=128))
            nc.sync.dma_start(bq_sb[:], bq.rearrange("(mb p) o -> p (mb o)", p=128))
            nc.sync.dma_start(mask_sb[:], mask_d[:])
            nc.sync.dma_start(cos_sb[:], cos_d[:])
            nc.sync.dma_start(sin_sb[:], sin_d[:])
            nc.sync.dma_start(wo_sb[:], woT.rearrange("(kc p) m -> p kc m", p=128))
            nc.sync.dma_start(bo_sb[:], bo[:])
            nc.gpsimd.partition_broadcast(bo_bc[:], bo_sb[:])
            nc.vector.memset(vaug_sb[:, :, D:D + 1], 1.0)

            dram = ctx.enter_context(tc.tile_pool(name="dram", bufs=1,
                                                  space="DRAM"))
            rows_a2a = 8 * DQ // 2
            a2a_in = [[dram.tile([rows_a2a, TS], BF16, name=f"a2a_in{b}_{hf}",
                                 tag=f"a2a_in{b}_{hf}") for hf in range(2)]
                      for b in range(B)]
            a2a_out = [[dram.tile([rows_a2a, TS], BF16, name=f"a2a_out{b}_{hf}",
                                  tag=f"a2a_out{b}_{hf}") for hf in range(2)]
                       for b in range(B)]
            v["a2a_in"], v["a2a_out"] = a2a_in, a2a_out
            v.update(wq_sb=wq_sb, wkv_sb=wkv_sb, bq_sb=bq_sb, bkv_sb=bkv_sb,
                     wo_sb=wo_sb, bo_bc=bo_bc, mask_sb=mask_sb, cos_sb=cos_sb,
                     sin_sb=sin_sb, qT_sb=qT_sb, kT_sb=kT_sb, vaug_sb=vaug_sb)

            for _ in range(repeat):
                _emit(tc, nc, v)
    nc.compile()
    return nc


def _emit(tc, nc, v):
    B, T, E, D, HQ = v["B"], v["T"], v["E"], v["D"], v["HQ"]
    BT, DQ, TS, KC, NB, QC = v["BT"], v["DQ"], v["TS"], v["KC"], v["NB"], v["QC"]
    xT, out = v["xT"], v["out"]
    wq_sb, wkv_sb, bq_sb, bkv_sb = v["wq_sb"], v["wkv_sb"], v["bq_sb"], v["bkv_sb"]
    wo_sb, bo_bc, mask_sb = v["wo_sb"], v["bo_bc"], v["mask_sb"]
    cos_sb, sin_sb = v["cos_sb"], v["sin_sb"]
    qT_sb, kT_sb, vaug_sb = v["qT_sb"], v["kT_sb"], v["vaug_sb"]
    a2a_in, a2a_out = v["a2a_in"], v["a2a_out"]

    with ExitStack() as ctx:
        psb = ctx.enter_context(tc.tile_pool(name="psb", bufs=2))
        asb = ctx.enter_context(tc.tile_pool(name="asb", bufs=2))
        aps = ctx.enter_context(tc.tile_pool(name="aps", bufs=2, space="PSUM"))
        p3sb = ctx.enter_context(tc.tile_pool(name="p3sb", bufs=1))
        # innermost pool: popped mid-emit (stack order) to hand its PSUM
        # banks to the o_proj pool
        pps_cm = tc.tile_pool(name="pps", bufs=2, space="PSUM")
        pps = pps_cm.__enter__()
        p3ps_box = [None]

        # ---- projection chunk: k|v + q for one 512-token slice, one x pass
        def proj_kv(nb):
            ns = slice(nb * 512, (nb + 1) * 512)
            xt = psb.tile([128, KC, 512], BF16, tag="xt", name="xt")
            nc.sync.dma_start(
                xt[:], xT[:, ns].rearrange("(kc p) n -> p kc n", p=128))
            pkv = pps.tile([128, 512], F32, tag="pp", name="pkv")
            for kc in range(KC):
                nc.tensor.matmul(pkv[:], wkv_sb[:, kc], xt[:, kc],
                                 start=(kc == 0), stop=(kc == KC - 1))
            kvf = psb.tile([128, 512], BF16, tag="kvf", name="kvf")
            nc.scalar.activation(kvf[:], pkv[:], AF.Identity, bias=bkv_sb[:])
            # rope on k (rows 0:64); sinm carries the rotate-half sign
            ksw = psb.tile([D, 512], BF16, tag="ksw", name="ksw")
            nc.sync.dma_start(ksw[0:32], kvf[32:64])
            nc.sync.dma_start(ksw[32:64], kvf[0:32])
            tk = psb.tile([D, 512], BF16, tag="tk", name="tk")
            nc.vector.tensor_mul(tk[:], kvf[0:D], cos_sb[0:D, ns])
            nc.vector.tensor_mul(ksw[:], ksw[:], sin_sb[0:D, ns])
            nc.vector.tensor_add(kT_sb[0:D, ns], tk[:], ksw[:])
            nc.sync.dma_start(kT_sb[D:128, ns], kT_sb[0:D, ns])
            # v rows 64:128 -> token-major via xbar transpose (no PE/PSUM).
            # The xbar needs a dense output region: land in staging, then one
            # strided DVE copy into vaug's 65-pitch layout.
            vstg = psb.tile([128, 4, D], BF16, tag="vstg", name="vstg")
            for i in range(4):
                nc.sync.dma_start_transpose(
                    vstg[:, i, :], kvf[D:128, i * 128:(i + 1) * 128])
            nc.vector.tensor_copy(vaug_sb[:, nb * 4:nb * 4 + 4, 0:D], vstg[:])
            return xt

        def proj_q(nb, xt, mb):
            ns = slice(nb * 512, (nb + 1) * 512)
            pq = pps.tile([128, 512], F32, tag="pp", name="pq")
            for kc in range(KC):
                nc.tensor.matmul(pq[:], wq_sb[:, kc, mb * 128:(mb + 1) * 128],
                                 xt[:, kc], start=(kc == 0), stop=(kc == KC - 1))
            qf = psb.tile([128, 512], BF16, tag="qf", name="qf")
            nc.vector.tensor_scalar_add(qf[:], pq[:], bq_sb[:, mb:mb + 1])
            qsw = psb.tile([128, 512], BF16, tag="qsw", name="qsw")
            for g in range(2):
                o = g * 64
                nc.sync.dma_start(qsw[o:o + 32], qf[o + 32:o + 64])
                nc.sync.dma_start(qsw[o + 32:o + 64], qf[o:o + 32])
            tq = psb.tile([128, 512], BF16, tag="tq", name="tq")
            nc.vector.tensor_mul(tq[:], qf[:], cos_sb[:, ns])
            nc.vector.tensor_mul(qsw[:], qsw[:], sin_sb[:, ns])
            nc.vector.tensor_add(qT_sb[:, mb, ns], tq[:], qsw[:])

        def proj_thunks(nb):
            box = [None]

            def t0():
                box[0] = proj_kv(nb)

            return [t0,
                    lambda: proj_q(nb, box[0], 0),
                    lambda: proj_q(nb, box[0], 1)]

        # ---- attention unit for (batch, 512-query chunk, head pair) ----
        def attn_hp_thunks(b, qc, hp):
            n_kb = 4 * qc + 4
            yacc_box = [None]

            def block(kb):
                def t():
                    off = max(0, (kb - 4 * qc) * 128)
                    ncols = 512 - off
                    qcol = b * T + qc * 512 + off
                    if kb == 0:
                        # one bank-sized accumulator per head; its 4 query
                        # subblocks share a single PSUM accumulation group
                        # (start zeroes the whole 2KB zero-region, so later
                        # first-touches of other slices overwrite correctly)
                        yacc_box[0] = [
                            aps.tile([128, 4, D + 1], F32, tag=f"yacc{i}",
                                     name=f"yacc{i}", bufs=1)
                            for i in range(2)]
                    yacc = yacc_box[0]
                    st = aps.tile([128, 2, 512], F32, tag="st", name="st")
                    for i in range(2):
                        po = i * D
                        nc.tensor.matmul(
                            st[:, i, :ncols],
                            kT_sb[po:po + D, b * T + kb * 128:b * T + (kb + 1) * 128],
                            qT_sb[po:po + D, hp, qcol:qcol + ncols],
                            start=True, stop=True)
                    pt = asb.tile([128, 2, 512], BF16, tag="pt", name="pt",
                                  bufs=3)
                    nc.scalar.activation(pt[:, :, :ncols], st[:, :, :ncols],
                                         AF.Exp)
                    diag = kb >= 4 * qc
                    if diag:
                        for i in range(2):
                            nc.vector.tensor_mul(pt[:, i, 0:128],
                                                 pt[:, i, 0:128], mask_sb[:])
                    s_min = max(0, kb - 4 * qc)
                    for i in range(2):
                        for s in range(s_min, 4):
                            colrel = s * 128 - off
                            nc.tensor.matmul(
                                yacc[i][:, s, :],
                                pt[:, i, colrel:colrel + 128],
                                vaug_sb[:, b * (T // 128) + kb, :],
                                start=(kb == 0 and s == 0),
                                stop=(kb == n_kb - 1 and s == 3))
                return t

            def epilogue():
                yacc = yacc_box[0]
                stg = asb.tile([128, 512], BF16, tag="stg", name="stg")
                for s in range(4):
                    yt = asb.tile([128, 2, D], BF16, tag="yt", name="yt")
                    for i in range(2):
                        r = asb.tile([128, 1], F32, tag="r", name="r")
                        nc.vector.reciprocal_approx_fast(
                            out=r[:], in_=yacc[i][:, s, D:D + 1])
                        nc.vector.tensor_scalar_mul(yt[:, i, :],
                                                    yacc[i][:, s, 0:D], r[:])
                    nc.sync.dma_start_transpose(
                        stg[:, s * 128:(s + 1) * 128],
                        yt.rearrange("p a b -> p (a b)"))
                for u in range(2):
                    j = 2 * qc + u
                    nc.sync.dma_start(
                        a2a_in[b][hp][j * 128:(j + 1) * 128, :],
                        stg[:, u * 256:(u + 1) * 256])

            return [block(kb) for kb in range(n_kb)] + [epilogue]

        def emit_a2a(b, hf):
            if v["no_collective"]:
                nc.sync.dma_start(a2a_out[b][hf][:], a2a_in[b][hf][:])
            else:
                nc.gpsimd.collective_compute(
                    "AllToAll", ALU.bypass,
                    replica_groups=[list(range(N_CORES))],
                    ins=[a2a_in[b][hf].opt()], outs=[a2a_out[b][hf].opt()])

        ya_tiles = {}

        def load_ya(b):
            # lo half first: its (even) kc chunks start accumulating while
            # the hi-half collective is still in flight
            yah = []
            for hf in range(2):
                ya = p3sb.tile([128, KC // 2, TS], BF16, tag=f"ya_{hf}",
                               name=f"ya_{hf}", bufs=1)
                src = a2a_out[b][hf].opt().rearrange("(kc p) t -> p kc t", p=128)
                for yi in range(2):
                    nc.sync.dma_start(ya[:, yi * 4:(yi + 1) * 4],
                                      src[:, yi * 4:(yi + 1) * 4])
                yah.append(ya)
            ya_tiles[b] = yah

        def oproj_unit(b, tb, oc):
            def t():
                yah = ya_tiles[b]
                kc_order = ([2 * i for i in range(KC // 2)]
                            + [2 * i + 1 for i in range(KC // 2)])
                ocs = slice(oc * 512, (oc + 1) * 512)
                po = p3ps_box[0].tile([128, 512], F32, tag="po", name="po")
                for ki, kc in enumerate(kc_order):
                    nc.tensor.matmul(po[:],
                                     yah[kc % 2][:, kc // 2,
                                                 tb * 128:(tb + 1) * 128],
                                     wo_sb[:, kc, ocs], start=(ki == 0),
                                     stop=(ki == KC - 1))
                osb = p3sb.tile([128, 512], BF16, tag="osb", name="osb", bufs=2)
                nc.vector.tensor_tensor(osb[:], po[:], bo_bc[:, ocs], ALU.add)
                nc.sync.dma_start(out[b * TS + tb * 128:b * TS + (tb + 1) * 128,
                                        ocs], osb[:])
            return t

        # ---- interleaved emission schedule ----
        def interleave(attn, fillers):
            # fillers spread through the attention thunk list, front-loaded
            n_a, n_f = len(attn), len(fillers)
            out_l = []
            fi = 0
            for ai, a in enumerate(attn):
                out_l.append(a)
                want = int((ai + 1) * n_f / n_a + 0.999) if n_a else n_f
                while fi < min(want, n_f):
                    out_l.append(fillers[fi])
                    fi += 1
            out_l.extend(fillers[fi:])
            return out_l

        def run(thunks):
            for t in thunks:
                t()

        # chunks 0..1 up front (attention b0 qc0 needs chunk 0; lead-in)
        run(proj_thunks(0))
        run(proj_thunks(1))
        # b0 attention interleaved with remaining b0 + b1 chunks 2..5
        for qc in range(QC):
            attn = attn_hp_thunks(0, qc, 0) + attn_hp_thunks(0, qc, 1)
            run(interleave(attn, proj_thunks(qc + 2)))
        emit_a2a(0, 0)
        emit_a2a(0, 1)
        # b1 attention: chunks 6..7, then ya0 load + b0 o_proj as fillers
        b0_units = [oproj_unit(0, tb, oc) for tb in range(2) for oc in range(4)]
        for qc in range(QC):
            attn = attn_hp_thunks(1, qc, 0) + attn_hp_thunks(1, qc, 1)
            if qc < 2:
                fillers = proj_thunks(qc + 6)
                if qc == 1:
                    fillers = fillers + [lambda: load_ya(0)]
                run(interleave(attn, fillers))
            else:
                if qc == 2:
                    # projection PSUM ring retires; o_proj PSUM opens
                    pps_cm.__exit__(None, None, None)
                    p3ps_box[0] = ctx.enter_context(
                        tc.tile_pool(name="p3ps", bufs=2, space="PSUM"))
                lo, hi = (0, 2) if qc == 2 else (2, 4)
                run(interleave(attn, b0_units[lo:hi]))
        emit_a2a(1, 0)
        emit_a2a(1, 1)
        # remaining b0 o_proj covers the A2A1 window, then b1 o_proj
        run(b0_units[4:8])
        load_ya(1)
        run([oproj_unit(1, tb, oc) for tb in range(2) for oc in range(4)])


# ---------------------------------------------------------------------------
# host-side sharding, execution, and gather
# ---------------------------------------------------------------------------
import numpy as np
import ml_dtypes

BF = ml_dtypes.bfloat16
ROPE_BASE = 10000.0
_CACHE = {}


def _rope_tables(T, D):
    inv_freq = 1.0 / (ROPE_BASE ** (np.arange(0, D, 2, dtype=np.float64) / D))
    t = np.arange(T, dtype=np.float64)
    freqs = np.einsum("i,j->ij", t, inv_freq)
    return np.cos(freqs), np.sin(freqs)


def _make_core_inputs(x, Wq, bq, Wk, bk, Wv, bv, Wo, bo):
    B, T, E = x.shape
    D = 64
    DQ = Wq.shape[0] // N_CORES
    BT = B * T
    scale = 1.0 / np.sqrt(D)

    xT = np.ascontiguousarray(x.reshape(BT, E).T).astype(BF)
    woT = np.ascontiguousarray(Wo.T).astype(BF)
    bo_row = bo.reshape(1, E).astype(BF)

    cos, sin = _rope_tables(T, D)
    cos32 = np.tile(cos.T, (1, B))
    sin32 = np.tile(sin.T, (1, B))
    cosf = np.tile(cos32, (4, 1)).astype(BF)
    sgn = np.where((np.arange(128) % 64) < 32, -1.0, 1.0)[:, None]
    sinm = (np.tile(sin32, (4, 1)) * sgn).astype(BF)

    k_idx, q_idx = np.meshgrid(np.arange(128), np.arange(128), indexing="ij")
    mask = (q_idx >= k_idx).astype(BF)

    maps = []
    for c in range(N_CORES):
        qs = slice(c * DQ, (c + 1) * DQ)
        ks = slice(c * D, (c + 1) * D)
        maps.append({
            "xT": xT,
            "wqT": np.ascontiguousarray((Wq[qs] * scale).T).astype(BF),
            "wkT": np.ascontiguousarray(Wk[ks].T).astype(BF),
            "wvT": np.ascontiguousarray(Wv[ks].T).astype(BF),
            "bq": (bq[qs] * scale).reshape(DQ, 1).astype(np.float32),
            "bkv": np.concatenate([bk[ks], bv[ks]]).reshape(128, 1).astype(np.float32),
            "woT": woT,
            "bo": bo_row,
            "cosf": cosf,
            "sinm": sinm,
            "mask": mask,
        })
    return maps


def kernel(x, Wq, bq, Wk, bk, Wv, bv, Wo, bo):
    from concourse import bass_utils

    x = np.asarray(x, dtype=np.float32)
    Wq, bq = np.asarray(Wq, np.float32), np.asarray(bq, np.float32)
    Wk, bk = np.asarray(Wk, np.float32), np.asarray(bk, np.float32)
    Wv, bv = np.asarray(Wv, np.float32), np.asarray(bv, np.float32)
    Wo, bo = np.asarray(Wo, np.float32), np.asarray(bo, np.float32)
    B, T, E = x.shape

    key = (B, T, E)
    if key not in _CACHE:
        _CACHE[key] = build(B=B, T=T, E=E)
    nc = _CACHE[key]

    maps = _make_core_inputs(x, Wq, bq, Wk, bk, Wv, bv, Wo, bo)
    res = bass_utils.run_bass_kernel_spmd(
        nc, maps, core_ids=list(range(N_CORES)))

    TS = T // N_CORES
    full = np.empty((B, T, E), dtype=np.float32)
    for c in range(N_CORES):
        o = res.results[c]["out"].astype(np.float32)
        for b in range(B):
            full[b, c * TS:(c + 1) * TS] = o[b * TS:(b + 1) * TS]
    return full
